# revision 5
# baseline (speedup 1.0000x reference)
"""DiversityLoss kernel for 8 Trainium2 NeuronCores.

Reference computes:
    loss = exp(mean(-D_img * D_noise))
where D_x[i,j] = (||x_i||^2 + ||x_j||^2 - 2 (X X^T)_ij) / d_x  for X in
{images, noises}.

The pairwise matrices never need to be materialized.  With
    a_i = ||img_i||^2, b_i = ||noise_i||^2, S1 = sum a, S2 = sum b,
    S3 = a.b, S4 = (Y^T a).(Y^T 1), S5 = (X^T b).(X^T 1), S6 = ||X^T Y||_F^2
the sum over all (i,j) of D_img*D_noise * (d_x*d_y) expands exactly to
    2*N*S3 + 2*S1*S2 - 4*S4 - 4*S5 + 4*S6
so   loss = exp(-(2*N*S3 + 2*S1*S2 - 4*S4 - 4*S5 + 4*S6) / (N^2 d_x d_y)).

Work split: S1..S5 are O(N*d) linear passes over data the host already
reads to quantize it; they are computed exactly on the host in fp64.  The
quadratic term S6 = ||X^T Y||_F^2 (99.5% of the FLOPs and all of the
memory-bound tensor traffic) runs on the 8 cores: the 12288 columns of X
are split 1536 per core, each core computes its slab of Z = Y^T X with
fp8 DoubleRow matmuls (2 MACs/cell/cycle, 256-row contraction per
matmul) and reduces sum(Z^2) on-chip; the host adds the 8 partial S6
values.  fp8 quantization of X and Y biases E[fp8(v)^2] by C_SQ
(computed exactly by integrating the normal density over the rounding
intervals), so S6 is divided by C_SQ^2.  Validated at ~1e-5 relative
error vs the fp32 reference.

Per-core device program:
  - DMA in: y8 = Y DoubleRow-interleaved [128, 16, 2, 256] fp8 (two
    pieces) and x = the core's X-column slab [128, 16, 2, 1536] fp8
    (eight pieces), interleaved across both HWDGE queues (sync +
    scalar) so transfers pipeline and matmuls start early.
  - Per row-pair q: 6 DR matmuls, stationary = a 128-column chunk of
    the Y pair-tile, moving = a 512-column slice of the x pair-tile,
    accumulating Z[yc][xc] in 6 PSUM banks (2 tiles x 3 banks) over all
    16 pairs.
  - Drains: sum-of-squares of the six [128, 512] PSUM banks, split
    across ScalarE (activation Square + accumulate), VectorE and
    GpSimd (copy + fused mult-reduce), into F [128, 3].
  - One tiny DMA out of F; host sums everything.
"""

import os
import sys

import numpy as np

for _p in ("/opt/trn_rl_repo", "/root/.axon_site/_ro/trn_rl_repo"):
    if os.path.isdir(_p) and _p not in sys.path:
        sys.path.append(_p)

import ml_dtypes

N = 4096
DX = 12288
DY = 256
NCORES = 8
KC = DX // NCORES        # 1536 X-columns per core
T = N // 128             # 32 row tiles of 128
Q = T // 2               # 16 DoubleRow pair-tiles

# E[fp8e4m3(v)^2] for v ~ N(0,1)  (exact; see module docstring)
C_SQ = 0.999275342216946

# x DMA pieces (pair ranges) and which queue issues them; the two HWDGE
# queues (sync, scalar) stream concurrently, so arrival roughly follows
# this interleaving.  Small final pieces shorten the tail.
X_CHUNKS_SCALAR = ((0, 2), (5, 9), (11, 14), (14, 15))
X_CHUNKS_SYNC = ((2, 5), (9, 11), (15, 16))

_PROG = None


def _build_program():
    from contextlib import ExitStack

    import concourse.bass as bass
    import concourse.tile as tile
    from concourse import bacc, mybir

    nc = bacc.Bacc(
        "TRN2",
        target_bir_lowering=False,
        debug=False,
        enable_asserts=False,
        num_devices=NCORES,
    )
    f32 = mybir.dt.float32
    bf16 = mybir.dt.bfloat16
    f8 = mybir.dt.float8e4
    DR = mybir.MatmulPerfMode.DoubleRow
    MULT = mybir.AluOpType.mult
    SQ = mybir.ActivationFunctionType.Square

    x = nc.dram_tensor("x", [128, Q, 2, KC], f8, kind="ExternalInput").ap()
    y8d = nc.dram_tensor("y8", [128, Q, 2, DY], f8, kind="ExternalInput").ap()
    f_out = nc.dram_tensor("f", [128, 3], f32, kind="ExternalOutput").ap()

    with tile.TileContext(nc) as tc, ExitStack() as ctx:
        data = ctx.enter_context(tc.tile_pool(name="data", bufs=1))
        scr = ctx.enter_context(tc.tile_pool(name="scr", bufs=1))
        zpsum = ctx.enter_context(tc.tile_pool(name="zpsum", bufs=1, space="PSUM"))

        M8 = data.tile([128, Q, 2, DY], f8, name="M8")
        xt = data.tile([128, Q, 2, KC], f8, name="xt")
        F = scr.tile([128, 3], f32, name="F")

        # input DMAs: y8 first (all matmuls need it), x pieces interleaved
        # across both queues.  Each piece gets a fresh completion semaphore,
        # so issues pipeline without reuse stalls.
        nc.sync.dma_start(M8[:, 0:8, :, :], y8d[:, 0:8, :, :])
        nc.scalar.dma_start(
            xt[:, 0:2, :, :], x[:, 0:2, :, :]
        )
        nc.sync.dma_start(M8[:, 8:Q, :, :], y8d[:, 8:Q, :, :])
        sc_i, sy_i = 1, 0
        order = []
        while sc_i < len(X_CHUNKS_SCALAR) or sy_i < len(X_CHUNKS_SYNC):
            if sy_i < len(X_CHUNKS_SYNC):
                order.append((nc.sync, X_CHUNKS_SYNC[sy_i]))
                sy_i += 1
            if sc_i < len(X_CHUNKS_SCALAR):
                order.append((nc.scalar, X_CHUNKS_SCALAR[sc_i]))
                sc_i += 1
        for eng, (q0, q1) in order:
            eng.dma_start(xt[:, q0:q1, :, :], x[:, q0:q1, :, :])

        # Z accumulators: one PSUM tile, 6 banks; each [128, 512] bank is
        # one matmul accumulation target (group g = yc*3 + xc).
        zAll = zpsum.tile([128, 6, 512], f32, name="zAll")

        for q in range(Q):
            for yc in range(2):
                for xc in range(3):
                    nc.tensor.matmul(
                        zAll[:, yc * 3 + xc, :],
                        lhsT=M8[:, q, :, yc * 128 : (yc + 1) * 128],
                        rhs=xt[:, q, :, xc * 512 : (xc + 1) * 512],
                        perf_mode=DR,
                        start=(q == 0),
                        stop=(q == Q - 1),
                    )

        # drains: sum(Z^2) -> F columns.  ScalarE squares 4 banks straight
        # out of PSUM; VectorE copies the last 2 banks to SBUF as bf16
        # (f32 rate) then runs the fused square-reduce at the 16-bit rate.
        sqA = scr.tile([128, 2048], bf16, name="sqA")
        nc.scalar.activation(sqA[:], zAll[:, 0:4, :], SQ, accum_out=F[:, 0:1])
        cB = scr.tile([128, 1024], bf16, name="cB")
        sqB = scr.tile([128, 1024], bf16, name="sqB")
        nc.vector.tensor_copy(cB[:], zAll[:, 4:6, :])
        nc.vector.scalar_tensor_tensor(
            out=sqB[:],
            in0=cB[:],
            scalar=1.0,
            in1=cB[:],
            op0=MULT,
            op1=MULT,
            accum_out=F[:, 1:2],
        )
        nc.vector.memset(F[:, 2:3], 0.0)

        nc.sync.dma_start(f_out, F[:])

    nc.compile()
    return nc


def _get_program():
    global _PROG
    if _PROG is None:
        _PROG = _build_program()
    return _PROG


_LAST_RESULTS = None


def kernel(noises: np.ndarray, images: np.ndarray) -> np.ndarray:
    from concourse import bass_utils

    global _LAST_RESULTS

    nc = _get_program()

    X = np.ascontiguousarray(images, dtype=np.float32).reshape(N, -1)
    Y = np.ascontiguousarray(noises, dtype=np.float32)

    # exact host-side terms (linear passes over data already being read)
    a = np.einsum("ij,ij->i", X, X, dtype=np.float64)
    b = np.einsum("ij,ij->i", Y, Y, dtype=np.float64)
    S1 = float(a.sum())
    S2 = float(b.sum())
    S3 = float(a @ b)
    Y64 = Y.astype(np.float64)
    S4 = float((Y64.T @ a) @ Y64.sum(axis=0))
    Xtb = X.T @ b.astype(np.float32)
    Xt1 = X.T @ np.ones(N, dtype=np.float32)
    S5 = float(Xtb.astype(np.float64) @ Xt1.astype(np.float64))

    x8 = X.astype(ml_dtypes.float8_e4m3)
    y8 = np.ascontiguousarray(
        Y.astype(ml_dtypes.float8_e4m3).reshape(Q, 2, 128, DY).transpose(2, 0, 1, 3)
    )

    in_maps = []
    for c in range(NCORES):
        xcore = np.ascontiguousarray(
            x8[:, c * KC : (c + 1) * KC].reshape(Q, 2, 128, KC).transpose(2, 0, 1, 3)
        )
        in_maps.append({"x": xcore, "y8": y8})

    res = bass_utils.run_bass_kernel_spmd(nc, in_maps, core_ids=list(range(NCORES)))
    _LAST_RESULTS = res

    S6 = 0.0
    for c in range(NCORES):
        Fc = np.asarray(res.results[c]["f"], dtype=np.float64)
        S6 += Fc.sum()
    S6 /= C_SQ * C_SQ

    num = 2.0 * N * S3 + 2.0 * S1 * S2 - 4.0 * S4 - 4.0 * S5 + 4.0 * S6
    mean = num / (float(N) * N * DX * DY)
    return np.asarray(np.exp(-mean), dtype=np.float32)


# revision 8
# speedup vs baseline: 1.2155x; 1.2155x over previous
"""DiversityLoss kernel for 8 Trainium2 NeuronCores.

Reference computes:
    loss = exp(mean(-D_img * D_noise))
where D_x[i,j] = (||x_i||^2 + ||x_j||^2 - 2 (X X^T)_ij) / d_x  for X in
{images, noises}.

The pairwise matrices never need to be materialized.  With
    a_i = ||img_i||^2, b_i = ||noise_i||^2, S1 = sum a, S2 = sum b,
    S3 = a.b, S4 = (Y^T a).(Y^T 1), S5 = (X^T b).(X^T 1), S6 = ||X^T Y||_F^2
the sum over all (i,j) of D_img*D_noise * (d_x*d_y) expands exactly to
    2*N*S3 + 2*S1*S2 - 4*S4 - 4*S5 + 4*S6
so   loss = exp(-(2*N*S3 + 2*S1*S2 - 4*S4 - 4*S5 + 4*S6) / (N^2 d_x d_y)).

Work split: S1..S5 are O(N*d) linear passes over data the host already
reads to quantize it; they are computed exactly on the host in fp64.  The
quadratic term S6 = ||X^T Y||_F^2 (99.5% of the FLOPs and all of the
memory-bound tensor traffic) runs on the 8 cores: the 12288 columns of X
are split 1536 per core, each core computes its slab of Z = Y^T X with
fp8 DoubleRow matmuls (2 MACs/cell/cycle, 256-row contraction per
matmul) and reduces sum(Z^2) on-chip; the host adds the 8 partial S6
values.  fp8 quantization of X and Y biases E[fp8(v)^2] by C_SQ
(computed exactly by integrating the normal density over the rounding
intervals), so S6 is divided by C_SQ^2.  Validated at ~1e-5 relative
error vs the fp32 reference.

Per-core device program:
  - DMA in: y8 = Y DoubleRow-interleaved [128, 16, 2, 256] fp8 (two
    pieces) and x = the core's X-column slab [128, 16, 2, 1536] fp8
    (eight pieces), interleaved across both HWDGE queues (sync +
    scalar) so transfers pipeline and matmuls start early.
  - Per row-pair q: 6 DR matmuls, stationary = a 128-column chunk of
    the Y pair-tile, moving = a 512-column slice of the x pair-tile,
    accumulating Z[yc][xc] in 6 PSUM banks (2 tiles x 3 banks) over all
    16 pairs.
  - Drains: sum-of-squares of the six [128, 512] PSUM banks, split
    across ScalarE (activation Square + accumulate), VectorE and
    GpSimd (copy + fused mult-reduce), into F [128, 3].
  - One tiny DMA out of F; host sums everything.
"""

import os
import sys

import numpy as np

for _p in ("/opt/trn_rl_repo", "/root/.axon_site/_ro/trn_rl_repo"):
    if os.path.isdir(_p) and _p not in sys.path:
        sys.path.append(_p)

import ml_dtypes

N = 4096
DX = 12288
DY = 256
NCORES = 8
KC = DX // NCORES        # 1536 X-columns per core
T = N // 128             # 32 row tiles of 128
Q = T // 2               # 16 DoubleRow pair-tiles

# E[fp8e4m3(v)^2] for v ~ N(0,1)  (exact; see module docstring)
C_SQ = 0.999275342216946

# DMA schedule: the two HWDGE queues (sync, scalar) stream concurrently
# and share the 16 DMA engines ~evenly, so per-queue byte totals must be
# balanced and each queue's FIFO must deliver its pairs in consumption
# order.  Each queue carries half of y8 first (133KB/pair-equivalent),
# then alternating x pair-chunks (393KB/pair); single-pair chunks at the
# start (early matmul start) and end (short tail).
X_CHUNKS_SYNC = ((0, 1), (4, 6), (8, 10), (12, 14), (15, 16))
X_CHUNKS_SCALAR = ((1, 2), (2, 4), (6, 8), (10, 12), (14, 15))

_PROG = None


def _build_program():
    from contextlib import ExitStack

    import concourse.bass as bass
    import concourse.tile as tile
    from concourse import bacc, mybir

    nc = bacc.Bacc(
        "TRN2",
        target_bir_lowering=False,
        debug=False,
        enable_asserts=False,
        num_devices=NCORES,
    )
    f32 = mybir.dt.float32
    bf16 = mybir.dt.bfloat16
    f8 = mybir.dt.float8e4
    DR = mybir.MatmulPerfMode.DoubleRow
    MULT = mybir.AluOpType.mult
    SQ = mybir.ActivationFunctionType.Square

    x = nc.dram_tensor("x", [128, Q, 2, KC], f8, kind="ExternalInput").ap()
    y8d = nc.dram_tensor("y8", [128, Q, 2, DY], f8, kind="ExternalInput").ap()
    f_out = nc.dram_tensor("f", [128, 2], f32, kind="ExternalOutput").ap()

    with tile.TileContext(nc) as tc, ExitStack() as ctx:
        data = ctx.enter_context(tc.tile_pool(name="data", bufs=1))
        scr = ctx.enter_context(tc.tile_pool(name="scr", bufs=1))
        zpsum = ctx.enter_context(tc.tile_pool(name="zpsum", bufs=1, space="PSUM"))

        M8 = data.tile([128, Q, 2, DY], f8, name="M8")
        xt = data.tile([128, Q, 2, KC], f8, name="xt")
        F = scr.tile([128, 2], f32, name="F")

        # input DMAs: half of y8 at the head of each queue, then x pairs
        # alternating.  Each piece gets a fresh completion semaphore, so
        # issues pipeline without reuse stalls.
        nc.sync.dma_start(M8[:, 0:8, :, :], y8d[:, 0:8, :, :])
        nc.scalar.dma_start(M8[:, 8:Q, :, :], y8d[:, 8:Q, :, :])
        for i in range(max(len(X_CHUNKS_SYNC), len(X_CHUNKS_SCALAR))):
            if i < len(X_CHUNKS_SYNC):
                q0, q1 = X_CHUNKS_SYNC[i]
                nc.sync.dma_start(xt[:, q0:q1, :, :], x[:, q0:q1, :, :])
            if i < len(X_CHUNKS_SCALAR):
                q0, q1 = X_CHUNKS_SCALAR[i]
                nc.scalar.dma_start(xt[:, q0:q1, :, :], x[:, q0:q1, :, :])

        # Z accumulators: separate PSUM tiles per drain engine so the two
        # drains have independent dependencies.  zA (4 banks) -> ScalarE,
        # zB (2 banks) -> VectorE.  Group (yc, xc): zA holds (0,0) (0,1)
        # (0,2) (1,0); zB holds (1,1) (1,2).
        zA = zpsum.tile([128, 4, 512], f32, name="zA")
        zB = zpsum.tile([128, 2, 512], f32, name="zB")

        def z_target(yc, xc):
            g = yc * 3 + xc
            return zA[:, g, :] if g < 4 else zB[:, g - 4, :]

        GORDER = [(0, 0), (0, 1), (0, 2), (1, 0), (1, 1), (1, 2)]
        # last pair: finish zB's groups first so VectorE's drain starts
        # while the zA groups are still streaming.
        GORDER_LAST = [(1, 1), (1, 2), (1, 0), (0, 0), (0, 1), (0, 2)]
        for q in range(Q):
            for yc, xc in GORDER_LAST if q == Q - 1 else GORDER:
                nc.tensor.matmul(
                    z_target(yc, xc),
                    lhsT=M8[:, q, :, yc * 128 : (yc + 1) * 128],
                    rhs=xt[:, q, :, xc * 512 : (xc + 1) * 512],
                    perf_mode=DR,
                    start=(q == 0),
                    stop=(q == Q - 1),
                )

        # drains: sum(Z^2) -> F columns.  ScalarE squares its 4 banks
        # straight out of PSUM; VectorE copies its 2 banks to SBUF as bf16
        # then runs the fused square-reduce.
        sqA = scr.tile([128, 2048], bf16, name="sqA")
        nc.scalar.activation(sqA[:], zA[:, :, :], SQ, accum_out=F[:, 0:1])
        cB = scr.tile([128, 1024], bf16, name="cB")
        sqB = scr.tile([128, 1024], bf16, name="sqB")
        nc.vector.tensor_copy(cB[:], zB[:, :, :])
        nc.vector.scalar_tensor_tensor(
            out=sqB[:],
            in0=cB[:],
            scalar=1.0,
            in1=cB[:],
            op0=MULT,
            op1=MULT,
            accum_out=F[:, 1:2],
        )

        nc.sync.dma_start(f_out, F[:])

    nc.compile()
    return nc


def _get_program():
    global _PROG
    if _PROG is None:
        _PROG = _build_program()
    return _PROG


_LAST_RESULTS = None


def kernel(noises: np.ndarray, images: np.ndarray) -> np.ndarray:
    from concourse import bass_utils

    global _LAST_RESULTS

    nc = _get_program()

    X = np.ascontiguousarray(images, dtype=np.float32).reshape(N, -1)
    Y = np.ascontiguousarray(noises, dtype=np.float32)

    # exact host-side terms (linear passes over data already being read)
    a = np.einsum("ij,ij->i", X, X, dtype=np.float64)
    b = np.einsum("ij,ij->i", Y, Y, dtype=np.float64)
    S1 = float(a.sum())
    S2 = float(b.sum())
    S3 = float(a @ b)
    Y64 = Y.astype(np.float64)
    S4 = float((Y64.T @ a) @ Y64.sum(axis=0))
    Xtb = X.T @ b.astype(np.float32)
    Xt1 = X.T @ np.ones(N, dtype=np.float32)
    S5 = float(Xtb.astype(np.float64) @ Xt1.astype(np.float64))

    x8 = X.astype(ml_dtypes.float8_e4m3)
    y8 = np.ascontiguousarray(
        Y.astype(ml_dtypes.float8_e4m3).reshape(Q, 2, 128, DY).transpose(2, 0, 1, 3)
    )

    in_maps = []
    for c in range(NCORES):
        xcore = np.ascontiguousarray(
            x8[:, c * KC : (c + 1) * KC].reshape(Q, 2, 128, KC).transpose(2, 0, 1, 3)
        )
        in_maps.append({"x": xcore, "y8": y8})

    res = bass_utils.run_bass_kernel_spmd(nc, in_maps, core_ids=list(range(NCORES)))
    _LAST_RESULTS = res

    S6 = 0.0
    for c in range(NCORES):
        Fc = np.asarray(res.results[c]["f"], dtype=np.float64)
        S6 += Fc.sum()
    S6 /= C_SQ * C_SQ

    num = 2.0 * N * S3 + 2.0 * S1 * S2 - 4.0 * S4 - 4.0 * S5 + 4.0 * S6
    mean = num / (float(N) * N * DX * DY)
    return np.asarray(np.exp(-mean), dtype=np.float32)


# revision 11
# speedup vs baseline: 1.3201x; 1.0860x over previous
"""DiversityLoss kernel for 8 Trainium2 NeuronCores.

Reference computes:
    loss = exp(mean(-D_img * D_noise))
where D_x[i,j] = (||x_i||^2 + ||x_j||^2 - 2 (X X^T)_ij) / d_x  for X in
{images, noises}.

The pairwise matrices never need to be materialized.  With
    a_i = ||img_i||^2, b_i = ||noise_i||^2, S1 = sum a, S2 = sum b,
    S3 = a.b, S4 = (Y^T a).(Y^T 1), S5 = (X^T b).(X^T 1), S6 = ||X^T Y||_F^2
the sum over all (i,j) of D_img*D_noise * (d_x*d_y) expands exactly to
    2*N*S3 + 2*S1*S2 - 4*S4 - 4*S5 + 4*S6
so   loss = exp(-(2*N*S3 + 2*S1*S2 - 4*S4 - 4*S5 + 4*S6) / (N^2 d_x d_y)).

Work split: S1..S5 are O(N*d) linear passes over data the host already
reads to quantize it; they are computed exactly on the host in fp64.  The
quadratic term S6 = ||X^T Y||_F^2 (99.5% of the FLOPs and all of the
memory-bound tensor traffic) runs on the 8 cores: the 12288 columns of X
are split 1536 per core, each core computes its slab of Z = Y^T X with
fp8 DoubleRow matmuls (256-row contraction per pass) and reduces
sum(Z^2) on-chip; the host adds the 8 partial S6 values.  fp8
quantization of X and Y biases E[fp8(v)^2] by C_SQ (computed exactly by
integrating the normal density over the rounding intervals), so S6 is
divided by C_SQ^2.  Validated at ~2.5e-4 relative error vs the fp32
reference (tolerance 2e-2).

Per-core device program:
  - One input tensor, pair-interleaved: chunk q holds the 256 Y columns
    of row-pair q followed by the core's 1536 X columns, so a single DMA
    stream delivers both operands in exactly consumption order.  Chunks
    alternate across both HWDGE queues (sync + scalar), single-pair at
    the head (early matmul start) and tail (short drain gate).
  - Warm-up matmuls on memset data start at t~0 so the PE p-state ramp
    (2.4 GHz after ~3us of continuous work) completes before real data
    lands; the 96-matmul stream (~216ns each) is the critical path.
  - Per row-pair q: 6 DR matmuls, stationary = a 128-column chunk of
    the Y pair-tile, moving = a 512-column slice of the x pair-tile,
    accumulating in 6 PSUM banks over all 16 pairs.
  - Drains: sum(Z^2): ScalarE squares 4 banks straight out of PSUM
    (activation Square + accumulate), VectorE copies 2 banks to SBUF
    and square-reduces; a ones-vector matmul folds the 128 partition
    partials into one partition so the output DMA is one descriptor.
"""

import os
import sys

import numpy as np

for _p in ("/opt/trn_rl_repo", "/root/.axon_site/_ro/trn_rl_repo"):
    if os.path.isdir(_p) and _p not in sys.path:
        sys.path.append(_p)

import ml_dtypes

N = 4096
DX = 12288
DY = 256
NCORES = 8
KC = DX // NCORES        # 1536 X-columns per core
W = DY + KC              # 1792 interleaved columns per pair
T = N // 128             # 32 row tiles of 128
Q = T // 2               # 16 DoubleRow pair-tiles

# E[fp8e4m3(v)^2] for v ~ N(0,1)  (exact; see module docstring)
C_SQ = 0.999275342216946

# pair-chunks per HWDGE queue: balanced bytes, global order ~ pair order,
# single-pair chunks at head and tail.
CHUNKS_SYNC = ((0, 1), (3, 4), (5, 7), (9, 11), (13, 14), (15, 16))
CHUNKS_SCALAR = ((1, 2), (2, 3), (4, 5), (7, 9), (11, 13), (14, 15))
WARMUP_MM = 14   # junk matmuls on memset data to pre-ramp the PE clock

_PROG = None


def _build_program():
    from contextlib import ExitStack

    import concourse.bass as bass
    import concourse.tile as tile
    from concourse import bacc, mybir

    nc = bacc.Bacc(
        "TRN2",
        target_bir_lowering=False,
        debug=False,
        enable_asserts=False,
        num_devices=NCORES,
    )
    f32 = mybir.dt.float32
    bf16 = mybir.dt.bfloat16
    f8 = mybir.dt.float8e4
    DR = mybir.MatmulPerfMode.DoubleRow
    MULT = mybir.AluOpType.mult
    SQ = mybir.ActivationFunctionType.Square

    xd = nc.dram_tensor("x", [128, Q, 2, W], f8, kind="ExternalInput").ap()
    f_out = nc.dram_tensor("f", [1, 2], f32, kind="ExternalOutput").ap()

    with tile.TileContext(nc) as tc, ExitStack() as ctx:
        data = ctx.enter_context(tc.tile_pool(name="data", bufs=1))
        scr = ctx.enter_context(tc.tile_pool(name="scr", bufs=1))
        zpsum = ctx.enter_context(tc.tile_pool(name="zpsum", bufs=1, space="PSUM"))

        XT = data.tile([128, Q, 2, W], f8, name="XT")
        F = scr.tile([128, 2], f32, name="F")
        wbuf = scr.tile([128, 2, 256], f8, name="wbuf")
        ones = scr.tile([128, 1], bf16, name="ones")
        Fb = scr.tile([128, 2], bf16, name="Fb")
        Fs = scr.tile([1, 2], f32, name="Fs")

        # warm-up constants, written by GpSimd right at kernel start
        nc.gpsimd.memset(wbuf[:], 0.0)
        nc.gpsimd.memset(ones[:], 1.0)

        # input DMAs: chunks alternate across both queues in pair order
        for i in range(max(len(CHUNKS_SYNC), len(CHUNKS_SCALAR))):
            if i < len(CHUNKS_SYNC):
                q0, q1 = CHUNKS_SYNC[i]
                nc.sync.dma_start(XT[:, q0:q1, :, :], xd[:, q0:q1, :, :])
            if i < len(CHUNKS_SCALAR):
                q0, q1 = CHUNKS_SCALAR[i]
                nc.scalar.dma_start(XT[:, q0:q1, :, :], xd[:, q0:q1, :, :])

        # Z accumulators: separate PSUM tiles per drain engine.  zA (4
        # banks) -> ScalarE, zB (2 banks) -> VectorE; zW is the warm-up
        # target, zF the partition-reduced output.
        zA = zpsum.tile([128, 4, 512], f32, name="zA")
        zB = zpsum.tile([128, 2, 512], f32, name="zB")
        zW = zpsum.tile([128, 512], f32, name="zW")
        zF = zpsum.tile([1, 2], f32, name="zF")

        # warm-up: keeps the PE busy (and its clock ramping) while the
        # first real chunks stream in
        for _ in range(WARMUP_MM):
            nc.tensor.matmul(
                zW[:, 0:256],
                lhsT=wbuf[:, :, 0:128],
                rhs=wbuf[:],
                perf_mode=DR,
                start=True,
                stop=True,
            )

        def z_target(yc, xc):
            g = yc * 3 + xc
            return zA[:, g, :] if g < 4 else zB[:, g - 4, :]

        GORDER = [(0, 0), (0, 1), (0, 2), (1, 0), (1, 1), (1, 2)]
        # last pair: finish zB's groups first so VectorE's drain starts
        # while the zA groups are still streaming.
        GORDER_LAST = [(1, 1), (1, 2), (1, 0), (0, 0), (0, 1), (0, 2)]
        for q in range(Q):
            for yc, xc in GORDER_LAST if q == Q - 1 else GORDER:
                nc.tensor.matmul(
                    z_target(yc, xc),
                    lhsT=XT[:, q, :, yc * 128 : (yc + 1) * 128],
                    rhs=XT[:, q, :, DY + xc * 512 : DY + (xc + 1) * 512],
                    perf_mode=DR,
                    start=(q == 0),
                    stop=(q == Q - 1),
                )

        # drains: sum(Z^2) -> F columns
        sqA = scr.tile([128, 2048], bf16, name="sqA")
        nc.scalar.activation(sqA[:], zA[:, :, :], SQ, accum_out=F[:, 0:1])
        cB = scr.tile([128, 1024], bf16, name="cB")
        sqB = scr.tile([128, 1024], bf16, name="sqB")
        nc.vector.tensor_copy(cB[:], zB[:, :, :])
        nc.vector.scalar_tensor_tensor(
            out=sqB[:],
            in0=cB[:],
            scalar=1.0,
            in1=cB[:],
            op0=MULT,
            op1=MULT,
            accum_out=F[:, 1:2],
        )

        # fold the 128 partition partials into one partition (ones-vector
        # matmul) so the output DMA is a single descriptor
        nc.vector.tensor_copy(Fb[:], F[:])
        nc.tensor.matmul(
            zF[:, :], lhsT=ones[:], rhs=Fb[:], start=True, stop=True
        )
        nc.vector.tensor_copy(Fs[:], zF[:, :])
        nc.sync.dma_start(f_out, Fs[:])

    nc.compile()
    return nc


def _get_program():
    global _PROG
    if _PROG is None:
        _PROG = _build_program()
    return _PROG


_LAST_RESULTS = None


def kernel(noises: np.ndarray, images: np.ndarray) -> np.ndarray:
    from concourse import bass_utils

    global _LAST_RESULTS

    nc = _get_program()

    X = np.ascontiguousarray(images, dtype=np.float32).reshape(N, -1)
    Y = np.ascontiguousarray(noises, dtype=np.float32)

    # exact host-side terms (linear passes over data already being read)
    a = np.einsum("ij,ij->i", X, X, dtype=np.float64)
    b = np.einsum("ij,ij->i", Y, Y, dtype=np.float64)
    S1 = float(a.sum())
    S2 = float(b.sum())
    S3 = float(a @ b)
    Y64 = Y.astype(np.float64)
    S4 = float((Y64.T @ a) @ Y64.sum(axis=0))
    Xtb = X.T @ b.astype(np.float32)
    Xt1 = X.T @ np.ones(N, dtype=np.float32)
    S5 = float(Xtb.astype(np.float64) @ Xt1.astype(np.float64))

    x8 = X.astype(ml_dtypes.float8_e4m3)
    y8 = Y.astype(ml_dtypes.float8_e4m3).reshape(Q, 2, 128, DY)

    in_maps = []
    for c in range(NCORES):
        xc = x8[:, c * KC : (c + 1) * KC].reshape(Q, 2, 128, KC)
        comb = np.empty((Q, 2, 128, W), dtype=ml_dtypes.float8_e4m3)
        comb[:, :, :, 0:DY] = y8
        comb[:, :, :, DY:W] = xc
        in_maps.append({"x": np.ascontiguousarray(comb.transpose(2, 0, 1, 3))})

    res = bass_utils.run_bass_kernel_spmd(nc, in_maps, core_ids=list(range(NCORES)))
    _LAST_RESULTS = res

    S6 = 0.0
    for c in range(NCORES):
        S6 += float(np.asarray(res.results[c]["f"], dtype=np.float64).sum())
    S6 /= C_SQ * C_SQ

    num = 2.0 * N * S3 + 2.0 * S1 * S2 - 4.0 * S4 - 4.0 * S5 + 4.0 * S6
    mean = num / (float(N) * N * DX * DY)
    return np.asarray(np.exp(-mean), dtype=np.float32)


# revision 12
# speedup vs baseline: 1.3910x; 1.0537x over previous
"""DiversityLoss kernel for 8 Trainium2 NeuronCores.

Reference computes:
    loss = exp(mean(-D_img * D_noise))
where D_x[i,j] = (||x_i||^2 + ||x_j||^2 - 2 (X X^T)_ij) / d_x  for X in
{images, noises}.

The pairwise matrices never need to be materialized.  With
    a_i = ||img_i||^2, b_i = ||noise_i||^2, S1 = sum a, S2 = sum b,
    S3 = a.b, S4 = (Y^T a).(Y^T 1), S5 = (X^T b).(X^T 1), S6 = ||X^T Y||_F^2
the sum over all (i,j) of D_img*D_noise * (d_x*d_y) expands exactly to
    2*N*S3 + 2*S1*S2 - 4*S4 - 4*S5 + 4*S6
so   loss = exp(-(2*N*S3 + 2*S1*S2 - 4*S4 - 4*S5 + 4*S6) / (N^2 d_x d_y)).

Work split: S1..S5 are O(N*d) linear passes over data the host already
reads to quantize it; they are computed exactly on the host in fp64.  The
quadratic term S6 = ||X^T Y||_F^2 (99.5% of the FLOPs and all of the
memory-bound tensor traffic) runs on the 8 cores: the 12288 columns of X
are split 1536 per core, each core computes its slab of Z = Y^T X with
fp8 DoubleRow matmuls (256-row contraction per pass) and reduces
sum(Z^2) on-chip; the host adds the 8 partial S6 values.  fp8
quantization of X and Y biases E[fp8(v)^2] by C_SQ (computed exactly by
integrating the normal density over the rounding intervals), so S6 is
divided by C_SQ^2.  Validated at ~2.5e-4 relative error vs the fp32
reference (tolerance 2e-2).

Per-core device program:
  - One input tensor, pair-interleaved: chunk q holds the 256 Y columns
    of row-pair q followed by the core's 1536 X columns, so a single DMA
    stream delivers both operands in exactly consumption order.  Chunks
    alternate across both HWDGE queues (sync + scalar), single-pair at
    the head (early matmul start) and tail (short drain gate).
  - Warm-up matmuls on memset data start at t~0 so the PE p-state ramp
    (2.4 GHz after ~3us of continuous work) completes before real data
    lands; the 96-matmul stream (~216ns each) is the critical path.
  - Per row-pair q: 6 DR matmuls, stationary = a 128-column chunk of
    the Y pair-tile, moving = a 512-column slice of the x pair-tile,
    accumulating in 6 PSUM banks over all 16 pairs.
  - Drains: sum(Z^2): ScalarE squares 4 banks straight out of PSUM
    (activation Square + accumulate), VectorE copies 2 banks to SBUF
    and square-reduces; a ones-vector matmul folds the 128 partition
    partials into one partition so the output DMA is one descriptor.
"""

import os
import sys

import numpy as np

for _p in ("/opt/trn_rl_repo", "/root/.axon_site/_ro/trn_rl_repo"):
    if os.path.isdir(_p) and _p not in sys.path:
        sys.path.append(_p)

import ml_dtypes

N = 4096
DX = 12288
DY = 256
NCORES = 8
KC = DX // NCORES        # 1536 X-columns per core
W = DY + KC              # 1792 interleaved columns per pair
T = N // 128             # 32 row tiles of 128
Q = T // 2               # 16 DoubleRow pair-tiles

# E[fp8e4m3(v)^2] for v ~ N(0,1)  (exact; see module docstring)
C_SQ = 0.999275342216946

# pair-chunks per HWDGE queue: balanced bytes, global order ~ pair order,
# single-pair chunks at head and tail.  The sync queue's DMA ring starts
# streaming ~1.8us before scalar's, so it carries the earliest pairs.
CHUNKS_SYNC = ((0, 1), (1, 2), (4, 5), (6, 8), (10, 12), (14, 15))
CHUNKS_SCALAR = ((2, 3), (3, 4), (5, 6), (8, 10), (12, 14), (15, 16))
WARMUP_MM = 18   # junk matmuls on memset data to pre-ramp the PE clock

_PROG = None


def _build_program():
    from contextlib import ExitStack

    import concourse.bass as bass
    import concourse.tile as tile
    from concourse import bacc, mybir

    nc = bacc.Bacc(
        "TRN2",
        target_bir_lowering=False,
        debug=False,
        enable_asserts=False,
        num_devices=NCORES,
    )
    f32 = mybir.dt.float32
    bf16 = mybir.dt.bfloat16
    f8 = mybir.dt.float8e4
    DR = mybir.MatmulPerfMode.DoubleRow
    MULT = mybir.AluOpType.mult
    SQ = mybir.ActivationFunctionType.Square

    xd = nc.dram_tensor("x", [128, Q, 2, W], f8, kind="ExternalInput").ap()
    f_out = nc.dram_tensor("f", [1, 2], f32, kind="ExternalOutput").ap()

    with tile.TileContext(nc) as tc, ExitStack() as ctx:
        data = ctx.enter_context(tc.tile_pool(name="data", bufs=1))
        scr = ctx.enter_context(tc.tile_pool(name="scr", bufs=1))
        zpsum = ctx.enter_context(tc.tile_pool(name="zpsum", bufs=1, space="PSUM"))

        XT = data.tile([128, Q, 2, W], f8, name="XT")
        F = scr.tile([128, 2], f32, name="F")
        wbuf = scr.tile([128, 2, 256], f8, name="wbuf")
        ones = scr.tile([128, 1], bf16, name="ones")
        Fb = scr.tile([128, 2], bf16, name="Fb")
        Fs = scr.tile([1, 2], f32, name="Fs")

        # warm-up constants, written by GpSimd right at kernel start
        nc.gpsimd.memset(wbuf[:], 0.0)
        nc.gpsimd.memset(ones[:], 1.0)

        # input DMAs: chunks alternate across both queues in pair order
        for i in range(max(len(CHUNKS_SYNC), len(CHUNKS_SCALAR))):
            if i < len(CHUNKS_SYNC):
                q0, q1 = CHUNKS_SYNC[i]
                nc.sync.dma_start(XT[:, q0:q1, :, :], xd[:, q0:q1, :, :])
            if i < len(CHUNKS_SCALAR):
                q0, q1 = CHUNKS_SCALAR[i]
                nc.scalar.dma_start(XT[:, q0:q1, :, :], xd[:, q0:q1, :, :])

        # Z accumulators: separate PSUM tiles per drain engine.  zA (4
        # banks) -> ScalarE, zB (2 banks) -> VectorE; zW is the warm-up
        # target, zF the partition-reduced output.
        zA = zpsum.tile([128, 4, 512], f32, name="zA")
        zB = zpsum.tile([128, 2, 512], f32, name="zB")
        zW = zpsum.tile([128, 512], f32, name="zW")
        zF = zpsum.tile([1, 2], f32, name="zF")

        # warm-up: keeps the PE busy (and its clock ramping) while the
        # first real chunks stream in
        for _ in range(WARMUP_MM):
            nc.tensor.matmul(
                zW[:, 0:256],
                lhsT=wbuf[:, :, 0:128],
                rhs=wbuf[:],
                perf_mode=DR,
                start=True,
                stop=True,
            )

        def z_target(yc, xc):
            g = yc * 3 + xc
            return zA[:, g, :] if g < 4 else zB[:, g - 4, :]

        GORDER = [(0, 0), (0, 1), (0, 2), (1, 0), (1, 1), (1, 2)]
        # last pair: finish zB's groups first so VectorE's drain starts
        # while the zA groups are still streaming.
        GORDER_LAST = [(1, 1), (1, 2), (1, 0), (0, 0), (0, 1), (0, 2)]
        for q in range(Q):
            for yc, xc in GORDER_LAST if q == Q - 1 else GORDER:
                nc.tensor.matmul(
                    z_target(yc, xc),
                    lhsT=XT[:, q, :, yc * 128 : (yc + 1) * 128],
                    rhs=XT[:, q, :, DY + xc * 512 : DY + (xc + 1) * 512],
                    perf_mode=DR,
                    start=(q == 0),
                    stop=(q == Q - 1),
                )

        # drains: sum(Z^2) -> F columns
        sqA = scr.tile([128, 2048], bf16, name="sqA")
        nc.scalar.activation(sqA[:], zA[:, :, :], SQ, accum_out=F[:, 0:1])
        cB = scr.tile([128, 1024], bf16, name="cB")
        sqB = scr.tile([128, 1024], bf16, name="sqB")
        nc.vector.tensor_copy(cB[:], zB[:, :, :])
        nc.vector.scalar_tensor_tensor(
            out=sqB[:],
            in0=cB[:],
            scalar=1.0,
            in1=cB[:],
            op0=MULT,
            op1=MULT,
            accum_out=F[:, 1:2],
        )

        # fold the 128 partition partials into one partition (ones-vector
        # matmul) so the output DMA is a single descriptor
        nc.vector.tensor_copy(Fb[:], F[:])
        nc.tensor.matmul(
            zF[:, :], lhsT=ones[:], rhs=Fb[:], start=True, stop=True
        )
        nc.vector.tensor_copy(Fs[:], zF[:, :])
        nc.sync.dma_start(f_out, Fs[:])

    nc.compile()
    return nc


def _get_program():
    global _PROG
    if _PROG is None:
        _PROG = _build_program()
    return _PROG


_LAST_RESULTS = None


def kernel(noises: np.ndarray, images: np.ndarray) -> np.ndarray:
    from concourse import bass_utils

    global _LAST_RESULTS

    nc = _get_program()

    X = np.ascontiguousarray(images, dtype=np.float32).reshape(N, -1)
    Y = np.ascontiguousarray(noises, dtype=np.float32)

    # exact host-side terms (linear passes over data already being read)
    a = np.einsum("ij,ij->i", X, X, dtype=np.float64)
    b = np.einsum("ij,ij->i", Y, Y, dtype=np.float64)
    S1 = float(a.sum())
    S2 = float(b.sum())
    S3 = float(a @ b)
    Y64 = Y.astype(np.float64)
    S4 = float((Y64.T @ a) @ Y64.sum(axis=0))
    Xtb = X.T @ b.astype(np.float32)
    Xt1 = X.T @ np.ones(N, dtype=np.float32)
    S5 = float(Xtb.astype(np.float64) @ Xt1.astype(np.float64))

    x8 = X.astype(ml_dtypes.float8_e4m3)
    y8 = Y.astype(ml_dtypes.float8_e4m3).reshape(Q, 2, 128, DY)

    in_maps = []
    for c in range(NCORES):
        xc = x8[:, c * KC : (c + 1) * KC].reshape(Q, 2, 128, KC)
        comb = np.empty((Q, 2, 128, W), dtype=ml_dtypes.float8_e4m3)
        comb[:, :, :, 0:DY] = y8
        comb[:, :, :, DY:W] = xc
        in_maps.append({"x": np.ascontiguousarray(comb.transpose(2, 0, 1, 3))})

    res = bass_utils.run_bass_kernel_spmd(nc, in_maps, core_ids=list(range(NCORES)))
    _LAST_RESULTS = res

    S6 = 0.0
    for c in range(NCORES):
        S6 += float(np.asarray(res.results[c]["f"], dtype=np.float64).sum())
    S6 /= C_SQ * C_SQ

    num = 2.0 * N * S3 + 2.0 * S1 * S2 - 4.0 * S4 - 4.0 * S5 + 4.0 * S6
    mean = num / (float(N) * N * DX * DY)
    return np.asarray(np.exp(-mean), dtype=np.float32)
